# revision 22
# baseline (speedup 1.0000x reference)
import sys

sys.path.insert(0, "/opt/trn_rl_repo")

import ml_dtypes
import numpy as np

import concourse.bass as bass
import concourse.mybir as mybir
import concourse.tile as tile
from concourse import bacc
from concourse.bass_utils import run_bass_kernel_spmd
from concourse.masks import make_identity

# Problem dims (hardcoded per harness contract)
N, S, C = 4096, 1, 512
E, H, V = 64, 512, 256
T_STEPS = 32
M = 8            # cores
NL = N // M      # 512 rows per core
P = 128
KH = H // P      # 4 k-tiles over hidden dim
MRZ = 2 * H // P  # 8 m-tiles over r,z gates
NB = NL // P     # 4 batch tiles per core
VB = V // P      # 2 tiles over vocab

F32 = mybir.dt.float32
F16 = mybir.dt.float16
BF16 = mybir.dt.bfloat16
SCALE = 2.0 ** 11      # fp16 lo parts pre-scaled by this
INV_SCALE = 2.0 ** -11

N2W = True       # n-gate: drop whhH@hLo pass (2-pass) if True
NPRE = 2         # A-phase tiles of step t+1 emitted before phase D of step t

_PROGRAM = None
LAST_RESULT = None


def _build_program():
    nc = bacc.Bacc("TRN2", target_bir_lowering=False, debug=False)

    whhH_d = nc.dram_tensor("whhH", [KH, P, 3 * H], F16, kind="ExternalInput")
    whhL_d = nc.dram_tensor("whhL", [KH, P, 3 * H], F16, kind="ExternalInput")
    wihEmbV1_d = nc.dram_tensor("wihEmbV1", [P, 3 * H], BF16, kind="ExternalInput")
    wihEmbV2_d = nc.dram_tensor("wihEmbV2", [P, 3 * H], BF16, kind="ExternalInput")
    embW_d = nc.dram_tensor("embW", [VB, P, P], BF16, kind="ExternalInput")
    fcWhH_d = nc.dram_tensor("fcWhH", [KH, P, V], F16, kind="ExternalInput")
    fcWhL_d = nc.dram_tensor("fcWhL", [KH, P, V], F16, kind="ExternalInput")
    fcWembV1_d = nc.dram_tensor("fcWembV1", [P, V], BF16, kind="ExternalInput")
    fcWembV2_d = nc.dram_tensor("fcWembV2", [P, V], BF16, kind="ExternalInput")
    Grz_d = nc.dram_tensor("Grz", [MRZ, P, NL], F32, kind="ExternalInput")
    GnHi_d = nc.dram_tensor("GnHi", [KH, P, NL], F16, kind="ExternalInput")
    GnLo_d = nc.dram_tensor("GnLo", [KH, P, NL], F16, kind="ExternalInput")
    LctxHi_d = nc.dram_tensor("LctxHi", [NB, P, V], F16, kind="ExternalInput")
    LctxLo_d = nc.dram_tensor("LctxLo", [NB, P, V], F16, kind="ExternalInput")
    embT0_d = nc.dram_tensor("embT0", [P, NL], BF16, kind="ExternalInput")
    bhhn_d = nc.dram_tensor("bhhn", [P, KH], F32, kind="ExternalInput")
    out_d = nc.dram_tensor("out", [NL, T_STEPS, V], F32, kind="ExternalOutput")

    Copy = mybir.ActivationFunctionType.Copy
    Ident = mybir.ActivationFunctionType.Identity
    Sig = mybir.ActivationFunctionType.Sigmoid
    Tanh = mybir.ActivationFunctionType.Tanh
    ADD = mybir.AluOpType.add
    MULT = mybir.AluOpType.mult

    with tile.TileContext(nc) as tc:
        with tc.tile_pool(name="const", bufs=1) as const, \
             tc.tile_pool(name="state", bufs=2) as state, \
             tc.tile_pool(name="work", bufs=3) as work, \
             tc.tile_pool(name="gate", bufs=1) as gate, \
             tc.tile_pool(name="outp", bufs=3) as outp, \
             tc.tile_pool(name="pbank", bufs=8, space="PSUM") as pbank:

            def bank(dtype=F32, cols=NL):
                t = pbank.tile([P, cols], dtype, tag="bank", name="bk",
                               padded_shape=[P, NL if dtype == F32 else 2 * NL])
                return t

            # ---- load constants ----
            identb = const.tile([P, P], BF16)
            make_identity(nc, identb)
            idf16 = const.tile([P, P], F16)
            make_identity(nc, idf16)
            idf16s = const.tile([P, P], F16)
            nc.scalar.activation(idf16s, idf16, Copy, 0.0, INV_SCALE)

            # whh is not needed until t=1: its DMAs go last so step 0 can
            # start as soon as the small tensors land
            wihEmbV1 = const.tile([P, 3 * H], BF16)
            nc.sync.dma_start(out=wihEmbV1, in_=wihEmbV1_d[:, :])
            wihEmbV2 = const.tile([P, 3 * H], BF16)
            nc.sync.dma_start(out=wihEmbV2, in_=wihEmbV2_d[:, :])
            embW = const.tile([P, VB, P], BF16)
            for k in range(VB):
                nc.sync.dma_start(out=embW[:, k, :], in_=embW_d[k])
            fcWhH = const.tile([P, KH, V], F16)
            fcWhL = const.tile([P, KH, V], F16)
            for k in range(KH):
                nc.sync.dma_start(out=fcWhH[:, k, :], in_=fcWhH_d[k])
                nc.sync.dma_start(out=fcWhL[:, k, :], in_=fcWhL_d[k])
            fcWembV1 = const.tile([P, V], BF16)
            nc.sync.dma_start(out=fcWembV1, in_=fcWembV1_d[:, :])
            fcWembV2 = const.tile([P, V], BF16)
            nc.sync.dma_start(out=fcWembV2, in_=fcWembV2_d[:, :])
            Grz = const.tile([P, MRZ, NL], F32)
            for m in range(MRZ):
                nc.sync.dma_start(out=Grz[:, m, :], in_=Grz_d[m])
            GnHi = const.tile([P, KH, NL], F16)
            GnLo = const.tile([P, KH, NL], F16)
            for k in range(KH):
                nc.sync.dma_start(out=GnHi[:, k, :], in_=GnHi_d[k])
                nc.sync.dma_start(out=GnLo[:, k, :], in_=GnLo_d[k])
            LctxHi = const.tile([P, NB, V], F16)
            LctxLo = const.tile([P, NB, V], F16)
            for nb in range(NB):
                nc.sync.dma_start(out=LctxHi[:, nb, :], in_=LctxHi_d[nb])
                nc.sync.dma_start(out=LctxLo[:, nb, :], in_=LctxLo_d[nb])
            bhhn = const.tile([P, KH], F32)
            nc.sync.dma_start(out=bhhn, in_=bhhn_d[:, :])

            embTs_cur = state.tile([P, NL], BF16, tag="embT")
            nc.sync.dma_start(out=embTs_cur, in_=embT0_d[:, :])

            whhH = const.tile([P, KH, 3 * H], F16)
            whhL = const.tile([P, KH, 3 * H], F16)
            for k in range(KH):
                nc.sync.dma_start(out=whhH[:, k, :], in_=whhH_d[k])
                nc.sync.dma_start(out=whhL[:, k, :], in_=whhL_d[k])

            def rz_mm_prefix(m, hHi_prev):
                """hh matmuls for r,z tile m (no emb): returns open pHi + pLo."""
                msl = slice(m * P, (m + 1) * P)
                pHi = bank()
                for k in range(KH):
                    nc.tensor.matmul(pHi, whhH[:, k, msl], hHi_prev[:, k, :],
                                     start=(k == 0), stop=False)
                pLo = bank()
                for k in range(KH):
                    nc.tensor.matmul(pLo, whhL[:, k, msl], hHi_prev[:, k, :],
                                     start=(k == 0), stop=(k == KH - 1))
                return pHi, pLo

            hHi_prev = None
            hLo_prev = None
            hT_prev = None
            pend = None  # A-prefix tiles for next step: list of (pHi, pLo)
            for t in range(T_STEPS):
                r_t = gate.tile([P, KH, NL], F32, tag="r")
                z_t = gate.tile([P, KH, NL], F32, tag="z")
                zp_t = gate.tile([P, KH, NL], F32, tag="zp")
                u_t = gate.tile([P, KH, NL], F32, tag="u")
                n_t = gate.tile([P, KH, NL], F32, tag="n")
                hT_cur = state.tile([P, KH, NL], F32, tag="h")
                hHi = state.tile([P, KH, NL], F16, tag="hHi")
                hLo = state.tile([P, KH, NL], F16, tag="hLo")

                # ---- gates r,z (wH@hH + wL@hH + emb; no hLo pass) ----
                for m in range(MRZ):
                    msl = slice(m * P, (m + 1) * P)
                    tmp = work.tile([P, NL], F32, tag="gtmp")
                    if t > 0:
                        if pend is not None and m < len(pend):
                            pHi, pLo = pend[m]
                        else:
                            pHi, pLo = rz_mm_prefix(m, hHi_prev)
                        nc.tensor.matmul(pHi, wihEmbV1[:, msl], embTs_cur,
                                         start=False, stop=False)
                        nc.tensor.matmul(pHi, wihEmbV2[:, msl], embTs_cur,
                                         start=False, stop=True)
                        tmpl = work.tile([P, NL], F32, tag="gtmpl")
                        nc.scalar.activation(tmpl, pLo, Copy, 0.0, INV_SCALE)
                        tmp0 = work.tile([P, NL], F32, tag="gtmp0")
                        nc.vector.tensor_add(tmp0, tmpl, pHi)
                        nc.gpsimd.tensor_add(tmp, tmp0, Grz[:, m, :])
                    else:
                        pHi = bank()
                        nc.tensor.matmul(pHi, wihEmbV1[:, msl], embTs_cur,
                                         start=True, stop=False)
                        nc.tensor.matmul(pHi, wihEmbV2[:, msl], embTs_cur,
                                         start=False, stop=True)
                        nc.vector.tensor_add(tmp, pHi, Grz[:, m, :])
                    if m < KH:
                        nc.scalar.activation(r_t[:, m, :], tmp, Sig)
                    else:
                        i = m - KH
                        nc.scalar.activation(z_t[:, i, :], tmp, Sig)
                        # zp = 1 - z (exact); u = z * h_prev, both off the
                        # critical path: consumed only after tanh(n)
                        nc.gpsimd.tensor_scalar(zp_t[:, i, :], z_t[:, i, :],
                                                -1.0, 1.0, MULT, ADD)
                        if t > 0:
                            nc.gpsimd.tensor_mul(u_t[:, i, :], z_t[:, i, :],
                                                 hT_prev[:, i, :])
                pend = None

                # ---- n gate + h update + h split ----
                for i in range(KH):
                    m = 2 * KH + i
                    msl = slice(m * P, (m + 1) * P)
                    if t > 0:
                        pHi = bank()
                        for k in range(KH):
                            nc.tensor.matmul(pHi, whhH[:, k, msl], hHi_prev[:, k, :],
                                             start=(k == 0), stop=(k == KH - 1))
                        pLo = bank()
                        for k in range(KH):
                            nc.tensor.matmul(pLo, whhL[:, k, msl], hHi_prev[:, k, :],
                                             start=(k == 0),
                                             stop=(N2W and k == KH - 1))
                        if not N2W:
                            for k in range(KH):
                                nc.tensor.matmul(pLo, whhH[:, k, msl],
                                                 hLo_prev[:, k, :],
                                                 start=False, stop=(k == KH - 1))
                    # pGx: Gctx_n (fp16 pair via identity) + emb contribution
                    pGx = bank()
                    nc.tensor.matmul(pGx, idf16, GnHi[:, i, :],
                                     start=True, stop=False)
                    nc.tensor.matmul(pGx, idf16s, GnLo[:, i, :],
                                     start=False, stop=False)
                    nc.tensor.matmul(pGx, wihEmbV1[:, msl], embTs_cur,
                                     start=False, stop=False)
                    nc.tensor.matmul(pGx, wihEmbV2[:, msl], embTs_cur,
                                     start=False, stop=True)
                    t2 = work.tile([P, NL], F32, tag="t2")
                    if t > 0:
                        t0 = work.tile([P, NL], F32, tag="t0")
                        nc.scalar.activation(t0, pLo, Ident, bhhn[:, i:i + 1],
                                             INV_SCALE)
                        t1 = work.tile([P, NL], F32, tag="t1")
                        nc.vector.tensor_add(t1, t0, pHi)
                        nc.vector.tensor_mul(t2, t1, r_t[:, i, :])
                    else:
                        nc.vector.tensor_scalar(t2, r_t[:, i, :], bhhn[:, i:i + 1],
                                                None, MULT)
                    nc.vector.tensor_add(t2, t2, pGx)
                    nc.scalar.activation(n_t[:, i, :], t2, Tanh)
                    # h = u + zp*n with u = z*h_prev precomputed in phase A;
                    # hHi produced by a direct fp16-output add (short tail)
                    t3 = work.tile([P, NL], F32, tag="t3")
                    nc.vector.tensor_mul(t3, zp_t[:, i, :], n_t[:, i, :])
                    if t > 0:
                        nc.vector.tensor_add(hHi[:, i, :], u_t[:, i, :], t3)
                        nc.gpsimd.tensor_add(hT_cur[:, i, :], u_t[:, i, :], t3)
                    else:
                        nc.vector.tensor_copy(hHi[:, i, :], t3)
                        nc.vector.tensor_copy(hT_cur[:, i, :], t3)
                    # split: hLo = (h - hHi) * 2^11 (off the hi critical path)
                    t4 = work.tile([P, NL], F32, tag="t4")
                    nc.scalar.activation(t4, hHi[:, i, :], Copy)
                    nc.vector.tensor_sub(t4, hT_cur[:, i, :], t4)
                    nc.vector.tensor_scalar(hLo[:, i, :], t4, SCALE, None, MULT)

                # ---- logits, k-interleaved so PE follows hHi/hLo production ----
                pls = [bank(cols=2 * V) for _ in range(NB)]
                for nb in range(NB):
                    nc.tensor.matmul(pls[nb][:, 0:V], idf16, LctxHi[:, nb, :],
                                     start=True, stop=False)
                    nsl = slice(nb * P, (nb + 1) * P)
                    nc.tensor.matmul(pls[nb][:, 0:V], embTs_cur[:, nsl], fcWembV1,
                                     start=False, stop=False)
                    nc.tensor.matmul(pls[nb][:, 0:V], embTs_cur[:, nsl], fcWembV2,
                                     start=False, stop=False)
                for k in range(KH):
                    for nb in range(NB):
                        nsl = slice(nb * P, (nb + 1) * P)
                        nc.tensor.matmul(pls[nb][:, 0:V], hHi[:, k, nsl],
                                         fcWhH[:, k, :], start=False,
                                         stop=(k == KH - 1))
                # lo groups start only after the hi group in the same bank
                # closed; hLo is fully available by then
                for nb in range(NB):
                    nc.tensor.matmul(pls[nb][:, V:2 * V], idf16, LctxLo[:, nb, :],
                                     start=True, stop=False)
                for k in range(KH):
                    for nb in range(NB):
                        nsl = slice(nb * P, (nb + 1) * P)
                        nc.tensor.matmul(pls[nb][:, V:2 * V], hLo[:, k, nsl],
                                         fcWhH[:, k, :], start=False, stop=False)
                        nc.tensor.matmul(pls[nb][:, V:2 * V], hHi[:, k, nsl],
                                         fcWhL[:, k, :], start=False,
                                         stop=(k == KH - 1))
                oh_nv = work.tile([P, NB, V], BF16, tag="ohnv")
                mx = work.tile([P, NB], F32, tag="mx")
                for nb in range(NB):
                    nsl = slice(nb * P, (nb + 1) * P)
                    pl = pls[nb]
                    lgl = work.tile([P, V], F32, tag="lgl")
                    nc.vector.tensor_scalar(lgl, pl[:, V:2 * V], INV_SCALE, None,
                                            MULT)
                    lg = outp.tile([P, V], F32, tag="lg")
                    nc.vector.tensor_add(lg, lgl, pl[:, 0:V])
                    if t < T_STEPS - 1:
                        nc.vector.tensor_reduce(out=mx[:, nb:nb + 1], in_=lg,
                                                axis=mybir.AxisListType.X,
                                                op=mybir.AluOpType.max)
                        nc.vector.tensor_scalar(oh_nv[:, nb, :], lg, mx[:, nb:nb + 1],
                                                None, mybir.AluOpType.is_equal)
                    nc.sync.dma_start(out=out_d[nsl, t, :], in_=lg)

                if t < T_STEPS - 1:
                    # A-phase hh prefix for step t+1 fills the PE while the
                    # one-hot -> embT chain completes
                    pend = [rz_mm_prefix(m, hHi) for m in range(NPRE)]
                    ohT = state.tile([P, VB, NL], BF16, tag="ohT")
                    for vb in range(VB):
                        pt = bank(dtype=BF16)
                        for nb in range(NB):
                            nc.tensor.transpose(pt[:, nb * P:(nb + 1) * P],
                                                oh_nv[:, nb, vb * P:(vb + 1) * P],
                                                identb)
                        nc.vector.tensor_copy(ohT[:, vb, :], pt)
                    embTs_next = state.tile([P, NL], BF16, tag="embT")
                    pe = bank()
                    for k in range(VB):
                        nc.tensor.matmul(pe, embW[:, k, :], ohT[:, k, :],
                                         start=(k == 0), stop=(k == VB - 1))
                    nc.vector.tensor_copy(embTs_next, pe)
                    embTs_cur = embTs_next

                hT_prev = hT_cur
                hHi_prev = hHi
                hLo_prev = hLo

    nc.compile()
    return nc


def _get_program():
    global _PROGRAM
    if _PROGRAM is None:
        _PROGRAM = _build_program()
    return _PROGRAM


def _split16(x):
    hi = x.astype(np.float16)
    lo = ((x - hi.astype(np.float32)) * SCALE).astype(np.float16)
    return hi, lo


def _splitbf(x):
    hi = x.astype(ml_dtypes.bfloat16)
    lo = (x - hi.astype(np.float32)).astype(ml_dtypes.bfloat16)
    return hi, lo


def kernel(encoded, init_token, emb_W, W_ih, W_hh, b_ih, b_hh, fc_W, fc_b, T):
    global LAST_RESULT
    assert int(T) == T_STEPS
    encoded = np.asarray(encoded, np.float32)
    init_token = np.asarray(init_token)
    emb_W = np.asarray(emb_W, np.float32)
    W_ih = np.asarray(W_ih, np.float32)
    W_hh = np.asarray(W_hh, np.float32)
    b_ih = np.asarray(b_ih, np.float32)
    b_hh = np.asarray(b_hh, np.float32)
    fc_W = np.asarray(fc_W, np.float32)
    fc_b = np.asarray(fc_b, np.float32)

    cx = np.ascontiguousarray

    whhT = W_hh.T  # [H, 3H]
    whhH, whhL = _split16(whhT)
    whhH = cx(whhH.reshape(KH, P, 3 * H))
    whhL = cx(whhL.reshape(KH, P, 3 * H))
    we_h, we_l = _splitbf(W_ih[:, :E].T)  # [E, 3H]
    wihEmbV1 = cx(np.concatenate([we_h, we_l], axis=0))  # [128, 3H]
    wihEmbV2 = cx(np.concatenate([we_l, we_h], axis=0))
    ew_h, ew_l = _splitbf(emb_W)  # [V, E]
    embW = cx(np.concatenate([ew_h, ew_l], axis=1).reshape(VB, P, P))  # [V,128]
    fh, fl = _split16(fc_W[:, E + C:].T)  # [H, V]
    fcWhH = cx(fh.reshape(KH, P, V))
    fcWhL = cx(fl.reshape(KH, P, V))
    fe_h, fe_l = _splitbf(fc_W[:, :E].T)  # [E, V]
    fcWembV1 = cx(np.concatenate([fe_h, fe_l], axis=0))
    fcWembV2 = cx(np.concatenate([fe_l, fe_h], axis=0))
    big = b_ih + b_hh
    big[2 * H:] = b_ih[2 * H:]
    bhhn = cx(b_hh[2 * H:].reshape(KH, P).T)

    ctx_all = encoded.reshape(N, C)
    tok_all = np.asarray(init_token).astype(np.int64)

    # host-side context preludes (fp32)
    WihCtxT = W_ih[:, E:].T  # [C, 3H]
    FcCtxT = fc_W[:, E:E + C].T  # [C, V]
    Gctx_all = (ctx_all @ WihCtxT + big).astype(np.float32)      # [N, 3H]
    Lctx_all = (ctx_all @ FcCtxT + fc_b).astype(np.float32)      # [N, V]
    eh_all = emb_W.astype(ml_dtypes.bfloat16).astype(np.float32)
    el_all = (emb_W - eh_all).astype(ml_dtypes.bfloat16).astype(np.float32)

    in_maps = []
    for c in range(M):
        sl = slice(c * NL, (c + 1) * NL)
        GcT = Gctx_all[sl].T  # [3H, NL]
        Grz = cx(GcT[:2 * H].reshape(MRZ, P, NL))
        GnHi, GnLo = _split16(GcT[2 * H:])
        GnHi = cx(GnHi.reshape(KH, P, NL))
        GnLo = cx(GnLo.reshape(KH, P, NL))
        Lc = Lctx_all[sl]  # [NL, V]
        LcHi, LcLo = _split16(Lc)
        LctxHi = cx(LcHi.reshape(NB, P, V))
        LctxLo = cx(LcLo.reshape(NB, P, V))
        toks = tok_all[sl]
        embT0 = cx(np.concatenate([eh_all[toks].T, el_all[toks].T], axis=0)
                   .astype(ml_dtypes.bfloat16))  # [128, NL]
        in_maps.append({
            "whhH": whhH, "whhL": whhL,
            "wihEmbV1": wihEmbV1, "wihEmbV2": wihEmbV2,
            "embW": embW, "fcWhH": fcWhH, "fcWhL": fcWhL,
            "fcWembV1": fcWembV1, "fcWembV2": fcWembV2,
            "Grz": Grz, "GnHi": GnHi, "GnLo": GnLo,
            "LctxHi": LctxHi, "LctxLo": LctxLo,
            "embT0": embT0, "bhhn": bhhn,
        })

    nc = _get_program()
    res = run_bass_kernel_spmd(nc, in_maps, core_ids=list(range(M)))
    LAST_RESULT = res
    out = np.empty((N, T_STEPS, V), np.float32)
    for c in range(M):
        out[c * NL:(c + 1) * NL] = res.results[c]["out"]
    return out


# revision 23
# speedup vs baseline: 1.0028x; 1.0028x over previous
import sys

sys.path.insert(0, "/opt/trn_rl_repo")

import ml_dtypes
import numpy as np

import concourse.bass as bass
import concourse.mybir as mybir
import concourse.tile as tile
from concourse import bacc
from concourse.bass_utils import run_bass_kernel_spmd
from concourse.masks import make_identity

# Problem dims (hardcoded per harness contract)
N, S, C = 4096, 1, 512
E, H, V = 64, 512, 256
T_STEPS = 32
M = 8            # cores
NL = N // M      # 512 rows per core
P = 128
KH = H // P      # 4 k-tiles over hidden dim
MRZ = 2 * H // P  # 8 m-tiles over r,z gates
NB = NL // P     # 4 batch tiles per core
VB = V // P      # 2 tiles over vocab

F32 = mybir.dt.float32
F16 = mybir.dt.float16
BF16 = mybir.dt.bfloat16
SCALE = 2.0 ** 11      # fp16 lo parts pre-scaled by this
INV_SCALE = 2.0 ** -11

N2W = True       # n-gate: drop whhH@hLo pass (2-pass) if True
NPRE = 2         # A-phase tiles of step t+1 emitted before phase D of step t

_PROGRAM = None
LAST_RESULT = None


def _build_program():
    nc = bacc.Bacc("TRN2", target_bir_lowering=False, debug=False)

    whhH_d = nc.dram_tensor("whhH", [KH, P, 3 * H], F16, kind="ExternalInput")
    whhL_d = nc.dram_tensor("whhL", [KH, P, 3 * H], F16, kind="ExternalInput")
    wihEmbV1_d = nc.dram_tensor("wihEmbV1", [P, 3 * H], BF16, kind="ExternalInput")
    wihEmbV2_d = nc.dram_tensor("wihEmbV2", [P, 3 * H], BF16, kind="ExternalInput")
    embW_d = nc.dram_tensor("embW", [VB, P, P], BF16, kind="ExternalInput")
    fcWhH_d = nc.dram_tensor("fcWhH", [KH, P, V], F16, kind="ExternalInput")
    fcWhL_d = nc.dram_tensor("fcWhL", [KH, P, V], F16, kind="ExternalInput")
    fcWembV1_d = nc.dram_tensor("fcWembV1", [P, V], BF16, kind="ExternalInput")
    fcWembV2_d = nc.dram_tensor("fcWembV2", [P, V], BF16, kind="ExternalInput")
    Grz_d = nc.dram_tensor("Grz", [MRZ, P, NL], F32, kind="ExternalInput")
    GnHi_d = nc.dram_tensor("GnHi", [KH, P, NL], F16, kind="ExternalInput")
    GnLo_d = nc.dram_tensor("GnLo", [KH, P, NL], F16, kind="ExternalInput")
    LctxHi_d = nc.dram_tensor("LctxHi", [NB, P, V], F16, kind="ExternalInput")
    LctxLo_d = nc.dram_tensor("LctxLo", [NB, P, V], F16, kind="ExternalInput")
    embT0_d = nc.dram_tensor("embT0", [P, NL], BF16, kind="ExternalInput")
    bhhn_d = nc.dram_tensor("bhhn", [P, KH], F32, kind="ExternalInput")
    out_d = nc.dram_tensor("out", [NL, T_STEPS, V], F32, kind="ExternalOutput")

    Copy = mybir.ActivationFunctionType.Copy
    Ident = mybir.ActivationFunctionType.Identity
    Sig = mybir.ActivationFunctionType.Sigmoid
    Tanh = mybir.ActivationFunctionType.Tanh
    ADD = mybir.AluOpType.add
    MULT = mybir.AluOpType.mult

    with tile.TileContext(nc) as tc:
        with tc.tile_pool(name="const", bufs=1) as const, \
             tc.tile_pool(name="state", bufs=2) as state, \
             tc.tile_pool(name="work", bufs=3) as work, \
             tc.tile_pool(name="gate", bufs=1) as gate, \
             tc.tile_pool(name="outp", bufs=3) as outp, \
             tc.tile_pool(name="pbank", bufs=8, space="PSUM") as pbank:

            def bank(dtype=F32, cols=NL):
                t = pbank.tile([P, cols], dtype, tag="bank", name="bk",
                               padded_shape=[P, NL if dtype == F32 else 2 * NL])
                return t

            # ---- load constants ----
            identb = const.tile([P, P], BF16)
            make_identity(nc, identb)
            idf16 = const.tile([P, P], F16)
            make_identity(nc, idf16)
            idf16s = const.tile([P, P], F16)
            nc.scalar.activation(idf16s, idf16, Copy, 0.0, INV_SCALE)

            # whh is not needed until t=1: its DMAs go last so step 0 can
            # start as soon as the small tensors land
            wihEmbV1 = const.tile([P, 3 * H], BF16)
            nc.sync.dma_start(out=wihEmbV1, in_=wihEmbV1_d[:, :])
            wihEmbV2 = const.tile([P, 3 * H], BF16)
            nc.sync.dma_start(out=wihEmbV2, in_=wihEmbV2_d[:, :])
            embW = const.tile([P, VB, P], BF16)
            for k in range(VB):
                nc.sync.dma_start(out=embW[:, k, :], in_=embW_d[k])
            fcWhH = const.tile([P, KH, V], F16)
            fcWhL = const.tile([P, KH, V], F16)
            for k in range(KH):
                nc.sync.dma_start(out=fcWhH[:, k, :], in_=fcWhH_d[k])
                nc.sync.dma_start(out=fcWhL[:, k, :], in_=fcWhL_d[k])
            fcWembV1 = const.tile([P, V], BF16)
            nc.sync.dma_start(out=fcWembV1, in_=fcWembV1_d[:, :])
            fcWembV2 = const.tile([P, V], BF16)
            nc.sync.dma_start(out=fcWembV2, in_=fcWembV2_d[:, :])
            Grz = const.tile([P, MRZ, NL], F32)
            for m in range(MRZ):
                nc.sync.dma_start(out=Grz[:, m, :], in_=Grz_d[m])
            GnHi = const.tile([P, KH, NL], F16)
            GnLo = const.tile([P, KH, NL], F16)
            for k in range(KH):
                nc.sync.dma_start(out=GnHi[:, k, :], in_=GnHi_d[k])
                nc.sync.dma_start(out=GnLo[:, k, :], in_=GnLo_d[k])
            LctxHi = const.tile([P, NB, V], F16)
            LctxLo = const.tile([P, NB, V], F16)
            for nb in range(NB):
                nc.sync.dma_start(out=LctxHi[:, nb, :], in_=LctxHi_d[nb])
                nc.sync.dma_start(out=LctxLo[:, nb, :], in_=LctxLo_d[nb])
            bhhn = const.tile([P, KH], F32)
            nc.sync.dma_start(out=bhhn, in_=bhhn_d[:, :])

            embTs_cur = state.tile([P, NL], BF16, tag="embT")
            nc.sync.dma_start(out=embTs_cur, in_=embT0_d[:, :])

            whhH = const.tile([P, KH, 3 * H], F16)
            whhL = const.tile([P, KH, 3 * H], F16)
            for k in range(KH):
                nc.sync.dma_start(out=whhH[:, k, :], in_=whhH_d[k])
                nc.sync.dma_start(out=whhL[:, k, :], in_=whhL_d[k])

            def rz_mm_prefix(m, hHi_prev):
                """hh matmuls for r,z tile m (no emb): returns open pHi + pLo."""
                msl = slice(m * P, (m + 1) * P)
                pHi = bank()
                for k in range(KH):
                    nc.tensor.matmul(pHi, whhH[:, k, msl], hHi_prev[:, k, :],
                                     start=(k == 0), stop=False)
                pLo = bank()
                for k in range(KH):
                    nc.tensor.matmul(pLo, whhL[:, k, msl], hHi_prev[:, k, :],
                                     start=(k == 0), stop=(k == KH - 1))
                return pHi, pLo

            hHi_prev = None
            hLo_prev = None
            hT_prev = None
            pend = None  # A-prefix tiles for next step: list of (pHi, pLo)
            for t in range(T_STEPS):
                r_t = gate.tile([P, KH, NL], F32, tag="r")
                z_t = gate.tile([P, KH, NL], F32, tag="z")
                zp_t = gate.tile([P, KH, NL], F32, tag="zp")
                u_t = gate.tile([P, KH, NL], F32, tag="u")
                n_t = gate.tile([P, KH, NL], F32, tag="n")
                hT_cur = state.tile([P, KH, NL], F32, tag="h")
                hHi = state.tile([P, KH, NL], F16, tag="hHi")
                hLo = state.tile([P, KH, NL], F16, tag="hLo")

                # ---- gates r,z (wH@hH + wL@hH + emb; no hLo pass) ----
                for m in range(MRZ):
                    msl = slice(m * P, (m + 1) * P)
                    tmp = work.tile([P, NL], F32, tag="gtmp")
                    if t > 0:
                        if pend is not None and m < len(pend):
                            pHi, pLo = pend[m]
                        else:
                            pHi, pLo = rz_mm_prefix(m, hHi_prev)
                        nc.tensor.matmul(pHi, wihEmbV1[:, msl], embTs_cur,
                                         start=False, stop=False)
                        nc.tensor.matmul(pHi, wihEmbV2[:, msl], embTs_cur,
                                         start=False, stop=True)
                        tmpl = work.tile([P, NL], F32, tag="gtmpl")
                        nc.scalar.activation(tmpl, pLo, Copy, 0.0, INV_SCALE)
                        tmp0 = work.tile([P, NL], F32, tag="gtmp0")
                        nc.vector.tensor_add(tmp0, tmpl, pHi)
                        nc.gpsimd.tensor_add(tmp, tmp0, Grz[:, m, :])
                    else:
                        pHi = bank()
                        nc.tensor.matmul(pHi, wihEmbV1[:, msl], embTs_cur,
                                         start=True, stop=False)
                        nc.tensor.matmul(pHi, wihEmbV2[:, msl], embTs_cur,
                                         start=False, stop=True)
                        nc.vector.tensor_add(tmp, pHi, Grz[:, m, :])
                    if m < KH:
                        nc.scalar.activation(r_t[:, m, :], tmp, Sig)
                    else:
                        i = m - KH
                        nc.scalar.activation(z_t[:, i, :], tmp, Sig)
                        # zp = 1 - z (exact); u = z * h_prev; computed early,
                        # consumed after tanh(n)
                        nc.vector.tensor_scalar(zp_t[:, i, :], z_t[:, i, :],
                                                -1.0, 1.0, MULT, ADD)
                        if t > 0:
                            nc.vector.tensor_mul(u_t[:, i, :], z_t[:, i, :],
                                                 hT_prev[:, i, :])
                pend = None

                # ---- n gate + h update + h split ----
                for i in range(KH):
                    m = 2 * KH + i
                    msl = slice(m * P, (m + 1) * P)
                    if t > 0:
                        pHi = bank()
                        for k in range(KH):
                            nc.tensor.matmul(pHi, whhH[:, k, msl], hHi_prev[:, k, :],
                                             start=(k == 0), stop=(k == KH - 1))
                        pLo = bank()
                        for k in range(KH):
                            nc.tensor.matmul(pLo, whhL[:, k, msl], hHi_prev[:, k, :],
                                             start=(k == 0),
                                             stop=(N2W and k == KH - 1))
                        if not N2W:
                            for k in range(KH):
                                nc.tensor.matmul(pLo, whhH[:, k, msl],
                                                 hLo_prev[:, k, :],
                                                 start=False, stop=(k == KH - 1))
                    # pGx: Gctx_n (fp16 pair via identity) + emb contribution
                    pGx = bank()
                    nc.tensor.matmul(pGx, idf16, GnHi[:, i, :],
                                     start=True, stop=False)
                    nc.tensor.matmul(pGx, idf16s, GnLo[:, i, :],
                                     start=False, stop=False)
                    nc.tensor.matmul(pGx, wihEmbV1[:, msl], embTs_cur,
                                     start=False, stop=False)
                    nc.tensor.matmul(pGx, wihEmbV2[:, msl], embTs_cur,
                                     start=False, stop=True)
                    t2 = work.tile([P, NL], F32, tag="t2")
                    if t > 0:
                        t0 = work.tile([P, NL], F32, tag="t0")
                        nc.scalar.activation(t0, pLo, Ident, bhhn[:, i:i + 1],
                                             INV_SCALE)
                        t1 = work.tile([P, NL], F32, tag="t1")
                        nc.vector.tensor_add(t1, t0, pHi)
                        nc.vector.tensor_mul(t2, t1, r_t[:, i, :])
                    else:
                        nc.vector.tensor_scalar(t2, r_t[:, i, :], bhhn[:, i:i + 1],
                                                None, MULT)
                    nc.vector.tensor_add(t2, t2, pGx)
                    nc.scalar.activation(n_t[:, i, :], t2, Tanh)
                    # h = u + zp*n with u = z*h_prev precomputed in phase A;
                    # hHi produced by a direct fp16-output add (short tail)
                    t3 = work.tile([P, NL], F32, tag="t3")
                    nc.vector.tensor_mul(t3, zp_t[:, i, :], n_t[:, i, :])
                    if t > 0:
                        nc.vector.tensor_add(hHi[:, i, :], u_t[:, i, :], t3)
                        nc.gpsimd.tensor_add(hT_cur[:, i, :], u_t[:, i, :], t3)
                    else:
                        nc.vector.tensor_copy(hHi[:, i, :], t3)
                        nc.vector.tensor_copy(hT_cur[:, i, :], t3)
                    # split: hLo = (h - hHi) * 2^11 (off the hi critical path)
                    t4 = work.tile([P, NL], F32, tag="t4")
                    nc.scalar.activation(t4, hHi[:, i, :], Copy)
                    nc.vector.tensor_sub(t4, hT_cur[:, i, :], t4)
                    nc.vector.tensor_scalar(hLo[:, i, :], t4, SCALE, None, MULT)

                # ---- logits, k-interleaved so PE follows hHi/hLo production ----
                pls = [bank(cols=2 * V) for _ in range(NB)]
                for nb in range(NB):
                    nc.tensor.matmul(pls[nb][:, 0:V], idf16, LctxHi[:, nb, :],
                                     start=True, stop=False)
                    nsl = slice(nb * P, (nb + 1) * P)
                    nc.tensor.matmul(pls[nb][:, 0:V], embTs_cur[:, nsl], fcWembV1,
                                     start=False, stop=False)
                    nc.tensor.matmul(pls[nb][:, 0:V], embTs_cur[:, nsl], fcWembV2,
                                     start=False, stop=False)
                for k in range(KH):
                    for nb in range(NB):
                        nsl = slice(nb * P, (nb + 1) * P)
                        nc.tensor.matmul(pls[nb][:, 0:V], hHi[:, k, nsl],
                                         fcWhH[:, k, :], start=False,
                                         stop=(k == KH - 1))
                # lo groups start only after the hi group in the same bank
                # closed; hLo is fully available by then
                for nb in range(NB):
                    nc.tensor.matmul(pls[nb][:, V:2 * V], idf16, LctxLo[:, nb, :],
                                     start=True, stop=False)
                for k in range(KH):
                    for nb in range(NB):
                        nsl = slice(nb * P, (nb + 1) * P)
                        nc.tensor.matmul(pls[nb][:, V:2 * V], hLo[:, k, nsl],
                                         fcWhH[:, k, :], start=False, stop=False)
                        nc.tensor.matmul(pls[nb][:, V:2 * V], hHi[:, k, nsl],
                                         fcWhL[:, k, :], start=False,
                                         stop=(k == KH - 1))
                oh_nv = work.tile([P, NB, V], BF16, tag="ohnv")
                mx = work.tile([P, NB], F32, tag="mx")
                for nb in range(NB):
                    nsl = slice(nb * P, (nb + 1) * P)
                    pl = pls[nb]
                    lgl = work.tile([P, V], F32, tag="lgl")
                    nc.vector.tensor_scalar(lgl, pl[:, V:2 * V], INV_SCALE, None,
                                            MULT)
                    lg = outp.tile([P, V], F32, tag="lg")
                    nc.vector.tensor_add(lg, lgl, pl[:, 0:V])
                    if t < T_STEPS - 1:
                        nc.vector.tensor_reduce(out=mx[:, nb:nb + 1], in_=lg,
                                                axis=mybir.AxisListType.X,
                                                op=mybir.AluOpType.max)
                        nc.vector.tensor_scalar(oh_nv[:, nb, :], lg, mx[:, nb:nb + 1],
                                                None, mybir.AluOpType.is_equal)
                    nc.sync.dma_start(out=out_d[nsl, t, :], in_=lg)

                if t < T_STEPS - 1:
                    # A-phase hh prefix for step t+1 fills the PE while the
                    # one-hot -> embT chain completes
                    pend = [rz_mm_prefix(m, hHi) for m in range(NPRE)]
                    ohT = state.tile([P, VB, NL], BF16, tag="ohT")
                    for vb in range(VB):
                        pt = bank(dtype=BF16)
                        for nb in range(NB):
                            nc.tensor.transpose(pt[:, nb * P:(nb + 1) * P],
                                                oh_nv[:, nb, vb * P:(vb + 1) * P],
                                                identb)
                        nc.vector.tensor_copy(ohT[:, vb, :], pt)
                    embTs_next = state.tile([P, NL], BF16, tag="embT")
                    pe = bank()
                    for k in range(VB):
                        nc.tensor.matmul(pe, embW[:, k, :], ohT[:, k, :],
                                         start=(k == 0), stop=(k == VB - 1))
                    nc.vector.tensor_copy(embTs_next, pe)
                    embTs_cur = embTs_next

                hT_prev = hT_cur
                hHi_prev = hHi
                hLo_prev = hLo

    nc.compile()
    return nc


def _get_program():
    global _PROGRAM
    if _PROGRAM is None:
        _PROGRAM = _build_program()
    return _PROGRAM


def _split16(x):
    hi = x.astype(np.float16)
    lo = ((x - hi.astype(np.float32)) * SCALE).astype(np.float16)
    return hi, lo


def _splitbf(x):
    hi = x.astype(ml_dtypes.bfloat16)
    lo = (x - hi.astype(np.float32)).astype(ml_dtypes.bfloat16)
    return hi, lo


def kernel(encoded, init_token, emb_W, W_ih, W_hh, b_ih, b_hh, fc_W, fc_b, T):
    global LAST_RESULT
    assert int(T) == T_STEPS
    encoded = np.asarray(encoded, np.float32)
    init_token = np.asarray(init_token)
    emb_W = np.asarray(emb_W, np.float32)
    W_ih = np.asarray(W_ih, np.float32)
    W_hh = np.asarray(W_hh, np.float32)
    b_ih = np.asarray(b_ih, np.float32)
    b_hh = np.asarray(b_hh, np.float32)
    fc_W = np.asarray(fc_W, np.float32)
    fc_b = np.asarray(fc_b, np.float32)

    cx = np.ascontiguousarray

    whhT = W_hh.T  # [H, 3H]
    whhH, whhL = _split16(whhT)
    whhH = cx(whhH.reshape(KH, P, 3 * H))
    whhL = cx(whhL.reshape(KH, P, 3 * H))
    we_h, we_l = _splitbf(W_ih[:, :E].T)  # [E, 3H]
    wihEmbV1 = cx(np.concatenate([we_h, we_l], axis=0))  # [128, 3H]
    wihEmbV2 = cx(np.concatenate([we_l, we_h], axis=0))
    ew_h, ew_l = _splitbf(emb_W)  # [V, E]
    embW = cx(np.concatenate([ew_h, ew_l], axis=1).reshape(VB, P, P))  # [V,128]
    fh, fl = _split16(fc_W[:, E + C:].T)  # [H, V]
    fcWhH = cx(fh.reshape(KH, P, V))
    fcWhL = cx(fl.reshape(KH, P, V))
    fe_h, fe_l = _splitbf(fc_W[:, :E].T)  # [E, V]
    fcWembV1 = cx(np.concatenate([fe_h, fe_l], axis=0))
    fcWembV2 = cx(np.concatenate([fe_l, fe_h], axis=0))
    big = b_ih + b_hh
    big[2 * H:] = b_ih[2 * H:]
    bhhn = cx(b_hh[2 * H:].reshape(KH, P).T)

    ctx_all = encoded.reshape(N, C)
    tok_all = np.asarray(init_token).astype(np.int64)

    # host-side context preludes (fp32)
    WihCtxT = W_ih[:, E:].T  # [C, 3H]
    FcCtxT = fc_W[:, E:E + C].T  # [C, V]
    Gctx_all = (ctx_all @ WihCtxT + big).astype(np.float32)      # [N, 3H]
    Lctx_all = (ctx_all @ FcCtxT + fc_b).astype(np.float32)      # [N, V]
    eh_all = emb_W.astype(ml_dtypes.bfloat16).astype(np.float32)
    el_all = (emb_W - eh_all).astype(ml_dtypes.bfloat16).astype(np.float32)

    in_maps = []
    for c in range(M):
        sl = slice(c * NL, (c + 1) * NL)
        GcT = Gctx_all[sl].T  # [3H, NL]
        Grz = cx(GcT[:2 * H].reshape(MRZ, P, NL))
        GnHi, GnLo = _split16(GcT[2 * H:])
        GnHi = cx(GnHi.reshape(KH, P, NL))
        GnLo = cx(GnLo.reshape(KH, P, NL))
        Lc = Lctx_all[sl]  # [NL, V]
        LcHi, LcLo = _split16(Lc)
        LctxHi = cx(LcHi.reshape(NB, P, V))
        LctxLo = cx(LcLo.reshape(NB, P, V))
        toks = tok_all[sl]
        embT0 = cx(np.concatenate([eh_all[toks].T, el_all[toks].T], axis=0)
                   .astype(ml_dtypes.bfloat16))  # [128, NL]
        in_maps.append({
            "whhH": whhH, "whhL": whhL,
            "wihEmbV1": wihEmbV1, "wihEmbV2": wihEmbV2,
            "embW": embW, "fcWhH": fcWhH, "fcWhL": fcWhL,
            "fcWembV1": fcWembV1, "fcWembV2": fcWembV2,
            "Grz": Grz, "GnHi": GnHi, "GnLo": GnLo,
            "LctxHi": LctxHi, "LctxLo": LctxLo,
            "embT0": embT0, "bhhn": bhhn,
        })

    nc = _get_program()
    res = run_bass_kernel_spmd(nc, in_maps, core_ids=list(range(M)))
    LAST_RESULT = res
    out = np.empty((N, T_STEPS, V), np.float32)
    for c in range(M):
        out[c * NL:(c + 1) * NL] = res.results[c]["out"]
    return out


# revision 27
# speedup vs baseline: 1.0343x; 1.0314x over previous
import sys

sys.path.insert(0, "/opt/trn_rl_repo")

import ml_dtypes
import numpy as np

import concourse.bass as bass
import concourse.mybir as mybir
import concourse.tile as tile
from concourse import bacc
from concourse.bass_utils import run_bass_kernel_spmd
from concourse.masks import make_identity

# Problem dims (hardcoded per harness contract)
N, S, C = 4096, 1, 512
E, H, V = 64, 512, 256
T_STEPS = 32
M = 8            # cores
NL = N // M      # 512 rows per core
P = 128
KH = H // P      # 4 k-tiles over hidden dim
MRZ = 2 * H // P  # 8 m-tiles over r,z gates
NB = NL // P     # 4 batch tiles per core
VB = V // P      # 2 tiles over vocab

F32 = mybir.dt.float32
F16 = mybir.dt.float16
BF16 = mybir.dt.bfloat16
SCALE = 2.0 ** 11      # fp16 lo parts pre-scaled by this
INV_SCALE = 2.0 ** -11

N2W = True       # n-gate: drop whhH@hLo pass (2-pass) if True
NPRE = 2         # A-phase tiles of step t+1 emitted before phase D of step t

_PROGRAM = None
LAST_RESULT = None


def _build_program():
    nc = bacc.Bacc("TRN2", target_bir_lowering=False, debug=False)

    whhH_d = nc.dram_tensor("whhH", [KH, P, 3 * H], F16, kind="ExternalInput")
    whhL_d = nc.dram_tensor("whhL", [KH, P, 3 * H], F16, kind="ExternalInput")
    wihEmbV1_d = nc.dram_tensor("wihEmbV1", [P, 3 * H], BF16, kind="ExternalInput")
    wihEmbV2_d = nc.dram_tensor("wihEmbV2", [P, 3 * H], BF16, kind="ExternalInput")
    embW_d = nc.dram_tensor("embW", [VB, P, P], BF16, kind="ExternalInput")
    fcWhH_d = nc.dram_tensor("fcWhH", [KH, P, V], F16, kind="ExternalInput")
    fcWhL_d = nc.dram_tensor("fcWhL", [KH, P, V], F16, kind="ExternalInput")
    fcWembV1_d = nc.dram_tensor("fcWembV1", [P, V], BF16, kind="ExternalInput")
    fcWembV2_d = nc.dram_tensor("fcWembV2", [P, V], BF16, kind="ExternalInput")
    Grz_d = nc.dram_tensor("Grz", [MRZ, P, NL], F32, kind="ExternalInput")
    GnHi_d = nc.dram_tensor("GnHi", [KH, P, NL], F16, kind="ExternalInput")
    GnLo_d = nc.dram_tensor("GnLo", [KH, P, NL], F16, kind="ExternalInput")
    LctxHi_d = nc.dram_tensor("LctxHi", [NB, P, V], F16, kind="ExternalInput")
    LctxLo_d = nc.dram_tensor("LctxLo", [NB, P, V], F16, kind="ExternalInput")
    embT0_d = nc.dram_tensor("embT0", [P, NL], BF16, kind="ExternalInput")
    bhhn_d = nc.dram_tensor("bhhn", [P, KH], F32, kind="ExternalInput")
    out_d = nc.dram_tensor("out", [NL, T_STEPS, V], F32, kind="ExternalOutput")

    Copy = mybir.ActivationFunctionType.Copy
    Ident = mybir.ActivationFunctionType.Identity
    Sig = mybir.ActivationFunctionType.Sigmoid
    Tanh = mybir.ActivationFunctionType.Tanh
    ADD = mybir.AluOpType.add
    MULT = mybir.AluOpType.mult

    with tile.TileContext(nc) as tc:
        with tc.tile_pool(name="const", bufs=1) as const, \
             tc.tile_pool(name="state", bufs=2) as state, \
             tc.tile_pool(name="work", bufs=3) as work, \
             tc.tile_pool(name="gate", bufs=1) as gate, \
             tc.tile_pool(name="outp", bufs=3) as outp, \
             tc.tile_pool(name="pbank", bufs=8, space="PSUM") as pbank:

            def bank(dtype=F32, cols=NL):
                t = pbank.tile([P, cols], dtype, tag="bank", name="bk",
                               padded_shape=[P, NL if dtype == F32 else 2 * NL])
                return t

            # ---- load constants ----
            identb = const.tile([P, P], BF16)
            make_identity(nc, identb)
            idf16 = const.tile([P, P], F16)
            make_identity(nc, idf16)
            idf16s = const.tile([P, P], F16)
            nc.scalar.activation(idf16s, idf16, Copy, 0.0, INV_SCALE)

            # whh is not needed until t=1: its DMAs go last so step 0 can
            # start as soon as the small tensors land
            wihEmbV1 = const.tile([P, 3 * H], BF16)
            nc.sync.dma_start(out=wihEmbV1, in_=wihEmbV1_d[:, :])
            wihEmbV2 = const.tile([P, 3 * H], BF16)
            nc.sync.dma_start(out=wihEmbV2, in_=wihEmbV2_d[:, :])
            embW = const.tile([P, VB, P], BF16)
            for k in range(VB):
                nc.sync.dma_start(out=embW[:, k, :], in_=embW_d[k])
            fcWhH = const.tile([P, KH, V], F16)
            fcWhL = const.tile([P, KH, V], F16)
            for k in range(KH):
                nc.sync.dma_start(out=fcWhH[:, k, :], in_=fcWhH_d[k])
                nc.sync.dma_start(out=fcWhL[:, k, :], in_=fcWhL_d[k])
            fcWembV1 = const.tile([P, V], BF16)
            nc.sync.dma_start(out=fcWembV1, in_=fcWembV1_d[:, :])
            fcWembV2 = const.tile([P, V], BF16)
            nc.sync.dma_start(out=fcWembV2, in_=fcWembV2_d[:, :])
            Grz = const.tile([P, MRZ, NL], F32)
            for m in range(MRZ):
                nc.sync.dma_start(out=Grz[:, m, :], in_=Grz_d[m])
            GnHi = const.tile([P, KH, NL], F16)
            GnLo = const.tile([P, KH, NL], F16)
            for k in range(KH):
                nc.sync.dma_start(out=GnHi[:, k, :], in_=GnHi_d[k])
                nc.sync.dma_start(out=GnLo[:, k, :], in_=GnLo_d[k])
            LctxHi = const.tile([P, NB, V], F16)
            LctxLo = const.tile([P, NB, V], F16)
            for nb in range(NB):
                nc.sync.dma_start(out=LctxHi[:, nb, :], in_=LctxHi_d[nb])
                nc.sync.dma_start(out=LctxLo[:, nb, :], in_=LctxLo_d[nb])
            bhhn = const.tile([P, KH], F32)
            nc.sync.dma_start(out=bhhn, in_=bhhn_d[:, :])

            embTs_cur = state.tile([P, NL], BF16, tag="embT")
            nc.sync.dma_start(out=embTs_cur, in_=embT0_d[:, :])

            whhH = const.tile([P, KH, 3 * H], F16)
            whhL = const.tile([P, KH, 3 * H], F16)
            for k in range(KH):
                nc.sync.dma_start(out=whhH[:, k, :], in_=whhH_d[k])
                nc.sync.dma_start(out=whhL[:, k, :], in_=whhL_d[k])

            def rz_mm_prefix(m, hHi_prev):
                """hh matmuls for r,z tile m (no emb): returns open pHi + pLo."""
                msl = slice(m * P, (m + 1) * P)
                pHi = bank()
                for k in range(KH):
                    nc.tensor.matmul(pHi, whhH[:, k, msl], hHi_prev[:, k, :],
                                     start=(k == 0), stop=False)
                pLo = bank()
                for k in range(KH):
                    nc.tensor.matmul(pLo, whhL[:, k, msl], hHi_prev[:, k, :],
                                     start=(k == 0), stop=(k == KH - 1))
                return pHi, pLo

            hHi_prev = None
            hLo_prev = None
            hT_prev = None
            pend = None  # A-prefix tiles for next step: list of (pHi, pLo)
            for t in range(T_STEPS):
                r_t = gate.tile([P, KH, NL], F32, tag="r")
                z_t = gate.tile([P, KH, NL], F32, tag="z")
                zp_t = gate.tile([P, KH, NL], F32, tag="zp")
                u_t = gate.tile([P, KH, NL], F32, tag="u")
                n_t = gate.tile([P, KH, NL], F32, tag="n")
                hT_cur = state.tile([P, KH, NL], F32, tag="h")
                hHi = state.tile([P, KH, NL], F16, tag="hHi")
                hLo = state.tile([P, KH, NL], F16, tag="hLo")

                # ---- gates r,z (wH@hH + wL@hH + emb; no hLo pass) ----
                for m in range(MRZ):
                    msl = slice(m * P, (m + 1) * P)
                    tmp = work.tile([P, NL], F32, tag="gtmp", bufs=2)
                    if t > 0:
                        if pend is not None and m < len(pend):
                            pHi, pLo = pend[m]
                        else:
                            pHi, pLo = rz_mm_prefix(m, hHi_prev)
                        nc.tensor.matmul(pHi, wihEmbV1[:, msl], embTs_cur,
                                         start=False, stop=False)
                        nc.tensor.matmul(pHi, wihEmbV2[:, msl], embTs_cur,
                                         start=False, stop=True)
                        tmpl = work.tile([P, NL], F32, tag="gtmpl", bufs=2)
                        nc.scalar.activation(tmpl, pLo, Copy, 0.0, INV_SCALE)
                        tmp0 = work.tile([P, NL], F32, tag="gtmp0", bufs=2)
                        nc.vector.tensor_add(tmp0, tmpl, pHi)
                        nc.gpsimd.tensor_add(tmp, tmp0, Grz[:, m, :])
                    else:
                        pHi = bank()
                        nc.tensor.matmul(pHi, wihEmbV1[:, msl], embTs_cur,
                                         start=True, stop=False)
                        nc.tensor.matmul(pHi, wihEmbV2[:, msl], embTs_cur,
                                         start=False, stop=True)
                        nc.vector.tensor_add(tmp, pHi, Grz[:, m, :])
                    if m < KH:
                        nc.scalar.activation(r_t[:, m, :], tmp, Sig)
                    else:
                        i = m - KH
                        nc.scalar.activation(z_t[:, i, :], tmp, Sig)
                        # zp = 1 - z (exact); u = z * h_prev; computed early,
                        # consumed after tanh(n)
                        nc.vector.tensor_scalar(zp_t[:, i, :], z_t[:, i, :],
                                                -1.0, 1.0, MULT, ADD)
                        if t > 0:
                            nc.vector.tensor_mul(u_t[:, i, :], z_t[:, i, :],
                                                 hT_prev[:, i, :])
                pend = None

                # ---- n gate + h update + h split ----
                for i in range(KH):
                    m = 2 * KH + i
                    msl = slice(m * P, (m + 1) * P)
                    if t > 0:
                        pHi = bank()
                        for k in range(KH):
                            nc.tensor.matmul(pHi, whhH[:, k, msl], hHi_prev[:, k, :],
                                             start=(k == 0), stop=(k == KH - 1))
                        pLo = bank()
                        for k in range(KH):
                            nc.tensor.matmul(pLo, whhL[:, k, msl], hHi_prev[:, k, :],
                                             start=(k == 0),
                                             stop=(N2W and k == KH - 1))
                        if not N2W:
                            for k in range(KH):
                                nc.tensor.matmul(pLo, whhH[:, k, msl],
                                                 hLo_prev[:, k, :],
                                                 start=False, stop=(k == KH - 1))
                    # pGx: Gctx_n (fp16 pair via identity) + emb contribution
                    pGx = bank()
                    nc.tensor.matmul(pGx, idf16, GnHi[:, i, :],
                                     start=True, stop=False)
                    nc.tensor.matmul(pGx, idf16s, GnLo[:, i, :],
                                     start=False, stop=False)
                    nc.tensor.matmul(pGx, wihEmbV1[:, msl], embTs_cur,
                                     start=False, stop=False)
                    nc.tensor.matmul(pGx, wihEmbV2[:, msl], embTs_cur,
                                     start=False, stop=True)
                    # combine + h-update chain processed in column halves so
                    # downstream (logits k-matmuls) unblock on the first half
                    t2 = work.tile([P, NL], F32, tag="t2", bufs=2)
                    t0 = work.tile([P, NL], F32, tag="t0", bufs=2)
                    t1 = work.tile([P, NL], F32, tag="t1", bufs=2)
                    t3 = work.tile([P, NL], F32, tag="t3", bufs=2)
                    t4 = work.tile([P, NL], F32, tag="t4", bufs=2)
                    for hs in (slice(0, NL // 2), slice(NL // 2, NL)):
                        if t > 0:
                            nc.scalar.activation(t0[:, hs], pLo[:, hs], Ident,
                                                 bhhn[:, i:i + 1], INV_SCALE)
                            nc.vector.tensor_add(t1[:, hs], t0[:, hs], pHi[:, hs])
                            nc.vector.tensor_mul(t2[:, hs], t1[:, hs],
                                                 r_t[:, i, hs])
                        else:
                            nc.vector.tensor_scalar(t2[:, hs], r_t[:, i, hs],
                                                    bhhn[:, i:i + 1], None, MULT)
                        nc.vector.tensor_add(t2[:, hs], t2[:, hs], pGx[:, hs])
                        nc.scalar.activation(n_t[:, i, hs], t2[:, hs], Tanh)
                        nc.vector.tensor_mul(t3[:, hs], zp_t[:, i, hs],
                                             n_t[:, i, hs])
                        if t > 0:
                            nc.vector.tensor_add(hHi[:, i, hs], u_t[:, i, hs],
                                                 t3[:, hs])
                        else:
                            nc.vector.tensor_copy(hHi[:, i, hs], t3[:, hs])
                    # fp32 h + hLo split, off the hi critical path, full width
                    if t > 0:
                        nc.vector.tensor_add(hT_cur[:, i, :], u_t[:, i, :], t3)
                    else:
                        nc.vector.tensor_copy(hT_cur[:, i, :], t3)
                    nc.scalar.activation(t4, hHi[:, i, :], Copy)
                    t5 = work.tile([P, NL], F32, tag="t5", bufs=2)
                    nc.gpsimd.tensor_sub(t5, hT_cur[:, i, :], t4)
                    nc.vector.tensor_scalar(hLo[:, i, :], t5, SCALE, None, MULT)

                # ---- logits, k-interleaved so PE follows hHi/hLo production ----
                pls = [bank(cols=2 * V) for _ in range(NB)]
                for nb in range(NB):
                    nc.tensor.matmul(pls[nb][:, 0:V], idf16, LctxHi[:, nb, :],
                                     start=True, stop=False)
                    nsl = slice(nb * P, (nb + 1) * P)
                    nc.tensor.matmul(pls[nb][:, 0:V], embTs_cur[:, nsl], fcWembV1,
                                     start=False, stop=False)
                    nc.tensor.matmul(pls[nb][:, 0:V], embTs_cur[:, nsl], fcWembV2,
                                     start=False, stop=False)
                for k in range(KH):
                    for nb in range(NB):
                        nsl = slice(nb * P, (nb + 1) * P)
                        nc.tensor.matmul(pls[nb][:, 0:V], hHi[:, k, nsl],
                                         fcWhH[:, k, :], start=False,
                                         stop=(k == KH - 1))
                # lo groups start only after the hi group in the same bank
                # closed; hLo is fully available by then
                for nb in range(NB):
                    nc.tensor.matmul(pls[nb][:, V:2 * V], idf16, LctxLo[:, nb, :],
                                     start=True, stop=False)
                for k in range(KH):
                    for nb in range(NB):
                        nsl = slice(nb * P, (nb + 1) * P)
                        nc.tensor.matmul(pls[nb][:, V:2 * V], hLo[:, k, nsl],
                                         fcWhH[:, k, :], start=False, stop=False)
                        nc.tensor.matmul(pls[nb][:, V:2 * V], hHi[:, k, nsl],
                                         fcWhL[:, k, :], start=False,
                                         stop=(k == KH - 1))
                oh_nv = work.tile([P, NB, V], BF16, tag="ohnv")
                mx = work.tile([P, NB], F32, tag="mx")
                for nb in range(NB):
                    nsl = slice(nb * P, (nb + 1) * P)
                    pl = pls[nb]
                    lgl = work.tile([P, V], F32, tag="lgl")
                    nc.vector.tensor_scalar(lgl, pl[:, V:2 * V], INV_SCALE, None,
                                            MULT)
                    lg = outp.tile([P, V], F32, tag="lg")
                    nc.vector.tensor_add(lg, lgl, pl[:, 0:V])
                    if t < T_STEPS - 1:
                        nc.vector.tensor_reduce(out=mx[:, nb:nb + 1], in_=lg,
                                                axis=mybir.AxisListType.X,
                                                op=mybir.AluOpType.max)
                        nc.vector.tensor_scalar(oh_nv[:, nb, :], lg, mx[:, nb:nb + 1],
                                                None, mybir.AluOpType.is_equal)
                    nc.sync.dma_start(out=out_d[nsl, t, :], in_=lg)

                if t < T_STEPS - 1:
                    # A-phase hh prefix for step t+1 fills the PE while the
                    # one-hot -> embT chain completes
                    pend = [rz_mm_prefix(m, hHi) for m in range(NPRE)]
                    ohT = state.tile([P, VB, NL], BF16, tag="ohT")
                    for vb in range(VB):
                        pt = bank(dtype=BF16)
                        for nb in range(NB):
                            nc.tensor.transpose(pt[:, nb * P:(nb + 1) * P],
                                                oh_nv[:, nb, vb * P:(vb + 1) * P],
                                                identb)
                        nc.vector.tensor_copy(ohT[:, vb, :], pt)
                    embTs_next = state.tile([P, NL], BF16, tag="embT")
                    pe = bank()
                    for k in range(VB):
                        nc.tensor.matmul(pe, embW[:, k, :], ohT[:, k, :],
                                         start=(k == 0), stop=(k == VB - 1))
                    nc.vector.tensor_copy(embTs_next, pe)
                    embTs_cur = embTs_next

                hT_prev = hT_cur
                hHi_prev = hHi
                hLo_prev = hLo

    nc.compile()
    return nc


def _get_program():
    global _PROGRAM
    if _PROGRAM is None:
        _PROGRAM = _build_program()
    return _PROGRAM


def _split16(x):
    hi = x.astype(np.float16)
    lo = ((x - hi.astype(np.float32)) * SCALE).astype(np.float16)
    return hi, lo


def _splitbf(x):
    hi = x.astype(ml_dtypes.bfloat16)
    lo = (x - hi.astype(np.float32)).astype(ml_dtypes.bfloat16)
    return hi, lo


def kernel(encoded, init_token, emb_W, W_ih, W_hh, b_ih, b_hh, fc_W, fc_b, T):
    global LAST_RESULT
    assert int(T) == T_STEPS
    encoded = np.asarray(encoded, np.float32)
    init_token = np.asarray(init_token)
    emb_W = np.asarray(emb_W, np.float32)
    W_ih = np.asarray(W_ih, np.float32)
    W_hh = np.asarray(W_hh, np.float32)
    b_ih = np.asarray(b_ih, np.float32)
    b_hh = np.asarray(b_hh, np.float32)
    fc_W = np.asarray(fc_W, np.float32)
    fc_b = np.asarray(fc_b, np.float32)

    cx = np.ascontiguousarray

    whhT = W_hh.T  # [H, 3H]
    whhH, whhL = _split16(whhT)
    whhH = cx(whhH.reshape(KH, P, 3 * H))
    whhL = cx(whhL.reshape(KH, P, 3 * H))
    we_h, we_l = _splitbf(W_ih[:, :E].T)  # [E, 3H]
    wihEmbV1 = cx(np.concatenate([we_h, we_l], axis=0))  # [128, 3H]
    wihEmbV2 = cx(np.concatenate([we_l, we_h], axis=0))
    ew_h, ew_l = _splitbf(emb_W)  # [V, E]
    embW = cx(np.concatenate([ew_h, ew_l], axis=1).reshape(VB, P, P))  # [V,128]
    fh, fl = _split16(fc_W[:, E + C:].T)  # [H, V]
    fcWhH = cx(fh.reshape(KH, P, V))
    fcWhL = cx(fl.reshape(KH, P, V))
    fe_h, fe_l = _splitbf(fc_W[:, :E].T)  # [E, V]
    fcWembV1 = cx(np.concatenate([fe_h, fe_l], axis=0))
    fcWembV2 = cx(np.concatenate([fe_l, fe_h], axis=0))
    big = b_ih + b_hh
    big[2 * H:] = b_ih[2 * H:]
    bhhn = cx(b_hh[2 * H:].reshape(KH, P).T)

    ctx_all = encoded.reshape(N, C)
    tok_all = np.asarray(init_token).astype(np.int64)

    # host-side context preludes (fp32)
    WihCtxT = W_ih[:, E:].T  # [C, 3H]
    FcCtxT = fc_W[:, E:E + C].T  # [C, V]
    Gctx_all = (ctx_all @ WihCtxT + big).astype(np.float32)      # [N, 3H]
    Lctx_all = (ctx_all @ FcCtxT + fc_b).astype(np.float32)      # [N, V]
    eh_all = emb_W.astype(ml_dtypes.bfloat16).astype(np.float32)
    el_all = (emb_W - eh_all).astype(ml_dtypes.bfloat16).astype(np.float32)

    in_maps = []
    for c in range(M):
        sl = slice(c * NL, (c + 1) * NL)
        GcT = Gctx_all[sl].T  # [3H, NL]
        Grz = cx(GcT[:2 * H].reshape(MRZ, P, NL))
        GnHi, GnLo = _split16(GcT[2 * H:])
        GnHi = cx(GnHi.reshape(KH, P, NL))
        GnLo = cx(GnLo.reshape(KH, P, NL))
        Lc = Lctx_all[sl]  # [NL, V]
        LcHi, LcLo = _split16(Lc)
        LctxHi = cx(LcHi.reshape(NB, P, V))
        LctxLo = cx(LcLo.reshape(NB, P, V))
        toks = tok_all[sl]
        embT0 = cx(np.concatenate([eh_all[toks].T, el_all[toks].T], axis=0)
                   .astype(ml_dtypes.bfloat16))  # [128, NL]
        in_maps.append({
            "whhH": whhH, "whhL": whhL,
            "wihEmbV1": wihEmbV1, "wihEmbV2": wihEmbV2,
            "embW": embW, "fcWhH": fcWhH, "fcWhL": fcWhL,
            "fcWembV1": fcWembV1, "fcWembV2": fcWembV2,
            "Grz": Grz, "GnHi": GnHi, "GnLo": GnLo,
            "LctxHi": LctxHi, "LctxLo": LctxLo,
            "embT0": embT0, "bhhn": bhhn,
        })

    nc = _get_program()
    res = run_bass_kernel_spmd(nc, in_maps, core_ids=list(range(M)))
    LAST_RESULT = res
    out = np.empty((N, T_STEPS, V), np.float32)
    for c in range(M):
        out[c * NL:(c + 1) * NL] = res.results[c]["out"]
    return out
